# revision 3
# baseline (speedup 1.0000x reference)
"""Bahdanau attention kernel for Trainium2, 8-core data-parallel over batch.

Reference computation (B=32, S=2048, D=U=1024):
    enc_p = h_enc @ W_enc + b_enc                      # [B,S,U]
    dec_p = h_dec @ W_dec + b_dec                      # [B,U]
    score = tanh(dec_p[:,None,:] + enc_p) @ W_com + b_com
    attn  = softmax(score, axis=1)                     # over S
    ctx   = sum(attn * enc_p, axis=1)                  # [B,U]

Device algorithm (per core, 4 batches):
  - enc projection computed TRANSPOSED: enc_p^T[u, s] tiles via
    matmul(lhsT=W_enc[d,u], rhs=h_enc^T[d,s]) in float32r (full-rate fp32).
  - tanh fused with per-partition bias z^T[u] = dec_p^T[u,b] + b_dec + b_enc.
  - score[s] = sum_u tanh(..)[u,s] * W_com[u] via matmul with W_com as lhsT.
  - b_com dropped: softmax is shift-invariant, and attn/ctx are its only uses.
  - softmax over S done on partition 0 ([1, 2048] row).
  - ctx uses the softmax identity sum_s attn_s (hW+b) = (sum_s attn_s h) W + b
    so enc_p never needs to be stored or recomputed: one matvec against
    h_enc (natural layout) + one [1,1024]x[1024,1024] projection.
h_enc is passed from the host in both natural and transposed layout (pure
layout change; all FLOPs happen on device).
"""

import numpy as np

from concourse import bacc, tile, mybir
from concourse.bass_utils import run_bass_kernel_spmd

N_CORES = 8
B, S, D, U = 32, 2048, 1024, 1024
BL = B // N_CORES  # batches per core
ND = D // 128      # d-tiles
NU = U // 128      # u-tiles
NS = S // 128      # s-tiles
SC = 512           # s-chunk (moving free dim) for the main matmul
NSC = S // SC

F32 = mybir.dt.float32
F32R = mybir.dt.float32r
AF = mybir.ActivationFunctionType
ALU = mybir.AluOpType
AX = mybir.AxisListType


def build_program():
    nc = bacc.Bacc("TRN2", target_bir_lowering=False, debug=False,
                   num_devices=N_CORES)

    hencT = nc.dram_tensor("hencT", [BL, ND, 128, S], F32R, kind="ExternalInput").ap()
    hencN = nc.dram_tensor("hencN", [BL, NS, 128, D], F32R, kind="ExternalInput").ap()
    wenc = nc.dram_tensor("wenc", [ND, 128, U], F32R, kind="ExternalInput").ap()
    wdec = nc.dram_tensor("wdec", [ND, 128, U], F32, kind="ExternalInput").ap()
    hdT = nc.dram_tensor("hdT", [128, ND, BL], F32, kind="ExternalInput").ap()
    bencT = nc.dram_tensor("bencT", [128, NU], F32, kind="ExternalInput").ap()
    bdecT = nc.dram_tensor("bdecT", [128, NU], F32, kind="ExternalInput").ap()
    bencN = nc.dram_tensor("bencN", [1, U], F32, kind="ExternalInput").ap()
    wcomT = nc.dram_tensor("wcomT", [128, NU], F32R, kind="ExternalInput").ap()

    ctx_out = nc.dram_tensor("ctx_out", [BL, U], F32, kind="ExternalOutput").ap()
    attn_out = nc.dram_tensor("attn_out", [BL, S], F32, kind="ExternalOutput").ap()

    with tile.TileContext(nc) as tc:
        with (
            tc.tile_pool(name="const", bufs=1) as constp,
            tc.tile_pool(name="hT", bufs=2) as hTp,
            tc.tile_pool(name="tT", bufs=3) as tTp,
            tc.tile_pool(name="hn", bufs=6) as hnp,
            tc.tile_pool(name="row", bufs=2) as rowp,
            tc.tile_pool(name="small", bufs=4) as smallp,
            tc.tile_pool(name="mm", bufs=2, space="PSUM") as mmp,
            tc.tile_pool(name="vec", bufs=3, space="PSUM") as vecp,
            tc.tile_pool(name="tp", bufs=2, space="PSUM") as tpp,
        ):
            # ---------- constants ----------
            wenc_sb = constp.tile([128, ND, U], F32R, tag="wenc")
            for k in range(ND):
                nc.sync.dma_start(out=wenc_sb[:, k, :], in_=wenc[k])
            wdec_sb = constp.tile([128, ND, U], F32, tag="wdec")
            for k in range(ND):
                nc.sync.dma_start(out=wdec_sb[:, k, :], in_=wdec[k])
            hdT_sb = constp.tile([128, ND, BL], F32, tag="hdT")
            nc.sync.dma_start(out=hdT_sb[:], in_=hdT[:])
            bencT_sb = constp.tile([128, NU], F32, tag="bencT")
            nc.sync.dma_start(out=bencT_sb[:], in_=bencT[:])
            bdecT_sb = constp.tile([128, NU], F32, tag="bdecT")
            nc.sync.dma_start(out=bdecT_sb[:], in_=bdecT[:])
            bencN_sb = constp.tile([1, U], F32, tag="bencN")
            nc.sync.dma_start(out=bencN_sb[:], in_=bencN[:])
            wcomT_sb = constp.tile([128, NU], F32R, tag="wcomT")
            nc.sync.dma_start(out=wcomT_sb[:], in_=wcomT[:])
            ones_sb = constp.tile([1, 1], F32, tag="ones")
            nc.vector.memset(ones_sb[:], 1.0)

            # ---------- dec projection: zT[u, j, b] = dec_p^T + b_dec + b_enc ----------
            zT_sb = constp.tile([128, NU, BL], F32, tag="zT")
            for j in range(NU):
                dp_ps = tpp.tile([128, BL], F32, tag="tp")
                for k in range(ND):
                    nc.tensor.matmul(
                        dp_ps[:],
                        wdec_sb[:, k, j * 128:(j + 1) * 128],
                        hdT_sb[:, k, :],
                        start=(k == 0), stop=(k == ND - 1),
                    )
                nc.vector.tensor_scalar(
                    zT_sb[:, j, :], dp_ps[:],
                    bdecT_sb[:, j:j + 1], bencT_sb[:, j:j + 1],
                    ALU.add, ALU.add,
                )

            for b in range(BL):
                # ---------- phase A: enc proj + tanh + score ----------
                score_sb = rowp.tile([1, S], F32, tag="row")
                for c in range(NSC):
                    hT_c = hTp.tile([128, ND, SC], F32R, tag="hT")
                    for k in range(ND):
                        nc.sync.dma_start(
                            out=hT_c[:, k, :],
                            in_=hencT[b, k, :, c * SC:(c + 1) * SC],
                        )
                    score_ps = vecp.tile([1, SC], F32, tag="vec")
                    for j in range(NU):
                        mm_ps = mmp.tile([128, SC], F32, tag="mm")
                        for k in range(ND):
                            nc.tensor.matmul(
                                mm_ps[:],
                                wenc_sb[:, k, j * 128:(j + 1) * 128],
                                hT_c[:, k, :],
                                start=(k == 0), stop=(k == ND - 1),
                            )
                        tT = tTp.tile([128, SC], F32R, tag="tT")
                        nc.scalar.activation(tT[:], mm_ps[:], AF.Tanh,
                                             bias=zT_sb[:, j, b:b + 1])
                        nc.tensor.matmul(
                            score_ps[:],
                            wcomT_sb[:, j:j + 1],
                            tT[:],
                            start=(j == 0), stop=(j == NU - 1),
                        )
                    nc.vector.tensor_copy(score_sb[:, c * SC:(c + 1) * SC], score_ps[:])

                # ---------- phase B: softmax over S (partition 0) ----------
                negmax = smallp.tile([1, 1], F32, tag="scalars")
                nc.vector.tensor_reduce(negmax[:], score_sb[:], AX.X, ALU.max,
                                        negate=True)
                exp_sb = rowp.tile([1, S], F32, tag="row")
                sumexp = smallp.tile([1, 1], F32, tag="scalars")
                nc.scalar.activation(exp_sb[:], score_sb[:], AF.Exp,
                                     bias=negmax[:], accum_out=sumexp[:])
                inv = smallp.tile([1, 1], F32, tag="scalars")
                nc.vector.reciprocal(inv[:], sumexp[:])
                attn_sb = rowp.tile([1, S], F32, tag="row")
                nc.vector.tensor_scalar(attn_sb[:], exp_sb[:], inv[:], None, ALU.mult)
                nc.sync.dma_start(out=attn_out[b:b + 1, :], in_=attn_sb[:])

                # ---------- phase C: exp row -> columns via K=1 matmul ----------
                acol_ps = tpp.tile([128, NS], F32, tag="tp")
                for i in range(NS):
                    nc.tensor.matmul(
                        acol_ps[:, i:i + 1],
                        exp_sb[:, i * 128:(i + 1) * 128],
                        ones_sb[:],
                        start=True, stop=True,
                    )
                acol_sb = smallp.tile([128, NS], F32R, tag="acol")
                nc.vector.tensor_copy(acol_sb[:], acol_ps[:])

                # ---------- phase D: ctx_pre = exp^T @ h ----------
                ctx_ps = [vecp.tile([1, 512], F32, tag="vec", name=f"ctx_ps{b}_{c2}")
                          for c2 in range(2)]
                for i in range(NS):
                    hn = hnp.tile([128, D], F32R, tag="hn")
                    nc.sync.dma_start(out=hn[:], in_=hencN[b, i])
                    for c2 in range(2):
                        nc.tensor.matmul(
                            ctx_ps[c2][:],
                            acol_sb[:, i:i + 1],
                            hn[:, c2 * 512:(c2 + 1) * 512],
                            start=(i == 0), stop=(i == NS - 1),
                        )

                # ---------- phase E: scale, transpose, final projection ----------
                ctxs_sb = rowp.tile([1, D], F32, tag="rowD")
                for c2 in range(2):
                    nc.vector.tensor_scalar(
                        ctxs_sb[:, c2 * 512:(c2 + 1) * 512], ctx_ps[c2][:],
                        inv[:], None, ALU.mult,
                    )
                ctxT_ps = tpp.tile([128, ND], F32, tag="tp")
                for j in range(ND):
                    nc.tensor.matmul(
                        ctxT_ps[:, j:j + 1],
                        ctxs_sb[:, j * 128:(j + 1) * 128],
                        ones_sb[:],
                        start=True, stop=True,
                    )
                ctxT_sb = smallp.tile([128, ND], F32R, tag="ctxT")
                nc.vector.tensor_copy(ctxT_sb[:], ctxT_ps[:])
                for c2 in range(2):
                    fin_ps = vecp.tile([1, 512], F32, tag="vec")
                    for j in range(ND):
                        nc.tensor.matmul(
                            fin_ps[:],
                            ctxT_sb[:, j:j + 1],
                            wenc_sb[:, j, c2 * 512:(c2 + 1) * 512],
                            start=(j == 0), stop=(j == ND - 1),
                        )
                    out_sb = smallp.tile([1, 512], F32, tag="outrow")
                    nc.vector.tensor_tensor(out_sb[:], fin_ps[:],
                                            bencN_sb[:, c2 * 512:(c2 + 1) * 512],
                                            ALU.add)
                    nc.sync.dma_start(out=ctx_out[b:b + 1, c2 * 512:(c2 + 1) * 512],
                                      in_=out_sb[:])

    nc.compile()
    return nc


_program = None


def _get_program():
    global _program
    if _program is None:
        _program = build_program()
    return _program


def make_in_maps(h_enc, h_dec, W_enc, b_enc, W_dec, b_dec, W_com, b_com):
    """Shard + layout-rearrange the full inputs into per-core input maps."""
    h_enc = np.asarray(h_enc, dtype=np.float32)
    h_dec = np.asarray(h_dec, dtype=np.float32)
    W_enc = np.asarray(W_enc, dtype=np.float32)
    b_enc = np.asarray(b_enc, dtype=np.float32)
    W_dec = np.asarray(W_dec, dtype=np.float32)
    b_dec = np.asarray(b_dec, dtype=np.float32)
    W_com = np.asarray(W_com, dtype=np.float32)

    wenc_a = np.ascontiguousarray(W_enc.reshape(ND, 128, U))
    wdec_a = np.ascontiguousarray(W_dec.reshape(ND, 128, U))
    bencT_a = np.ascontiguousarray(b_enc.reshape(NU, 128).T)
    bdecT_a = np.ascontiguousarray(b_dec.reshape(NU, 128).T)
    bencN_a = np.ascontiguousarray(b_enc.reshape(1, U))
    wcomT_a = np.ascontiguousarray(W_com[:, 0].reshape(NU, 128).T)

    in_maps = []
    for core in range(N_CORES):
        hb = h_enc[core * BL:(core + 1) * BL]            # [BL, S, D]
        hdb = h_dec[core * BL:(core + 1) * BL]           # [BL, D]
        hencT_a = np.ascontiguousarray(
            hb.transpose(0, 2, 1).reshape(BL, ND, 128, S))
        hencN_a = np.ascontiguousarray(hb.reshape(BL, NS, 128, D))
        hdT_a = np.ascontiguousarray(
            hdb.reshape(BL, ND, 128).transpose(2, 1, 0))  # [128, ND, BL]
        in_maps.append({
            "hencT": hencT_a,
            "hencN": hencN_a,
            "wenc": wenc_a,
            "wdec": wdec_a,
            "hdT": hdT_a,
            "bencT": bencT_a,
            "bdecT": bdecT_a,
            "bencN": bencN_a,
            "wcomT": wcomT_a,
        })
    return in_maps


def kernel(h_enc, h_dec, W_enc, b_enc, W_dec, b_dec, W_com, b_com,
           _trace=False, _tmpdir=None):
    nc = _get_program()
    in_maps = make_in_maps(h_enc, h_dec, W_enc, b_enc, W_dec, b_dec,
                           W_com, b_com)
    res = run_bass_kernel_spmd(nc, in_maps, list(range(N_CORES)),
                               trace=_trace, tmpdir=_tmpdir)
    ctx = np.concatenate([res.results[i]["ctx_out"] for i in range(N_CORES)],
                         axis=0)
    attn = np.concatenate([res.results[i]["attn_out"] for i in range(N_CORES)],
                          axis=0).reshape(B, S, 1)
    if _trace:
        kernel.last_results = res
    return ctx.astype(np.float32), attn.astype(np.float32)


# revision 4
# speedup vs baseline: 1.0249x; 1.0249x over previous
"""Bahdanau attention kernel for Trainium2, 8-core data-parallel over batch.

Reference computation (B=32, S=2048, D=U=1024):
    enc_p = h_enc @ W_enc + b_enc                      # [B,S,U]
    dec_p = h_dec @ W_dec + b_dec                      # [B,U]
    score = tanh(dec_p[:,None,:] + enc_p) @ W_com + b_com
    attn  = softmax(score, axis=1)                     # over S
    ctx   = sum(attn * enc_p, axis=1)                  # [B,U]

Device algorithm (per core, 4 batches):
  - enc projection computed TRANSPOSED: enc_p^T[u, s] tiles via
    matmul(lhsT=W_enc[d,u], rhs=h_enc^T[d,s]) in float32r (full-rate fp32).
  - tanh fused with per-partition bias z^T[u] = dec_p^T[u,b] + b_dec + b_enc.
  - score[s] = sum_u tanh(..)[u,s] * W_com[u] via matmul with W_com as lhsT.
  - b_com dropped: softmax is shift-invariant, and attn/ctx are its only uses.
  - softmax over S done on partition 0 ([1, 2048] row).
  - ctx uses the softmax identity sum_s attn_s (hW+b) = (sum_s attn_s h) W + b
    so enc_p never needs to be stored or recomputed: one matvec against
    h_enc (natural layout) + one [1,1024]x[1024,1024] projection.
h_enc is passed from the host in both natural and transposed layout (pure
layout change; all FLOPs happen on device).
"""

import numpy as np

from concourse import bacc, tile, mybir
from concourse.bass_utils import run_bass_kernel_spmd

N_CORES = 8
B, S, D, U = 32, 2048, 1024, 1024
BL = B // N_CORES  # batches per core
ND = D // 128      # d-tiles
NU = U // 128      # u-tiles
NS = S // 128      # s-tiles
SC = 512           # s-chunk (moving free dim) for the main matmul
NSC = S // SC

F32 = mybir.dt.float32
F32R = mybir.dt.float32r
AF = mybir.ActivationFunctionType
ALU = mybir.AluOpType
AX = mybir.AxisListType


def build_program():
    nc = bacc.Bacc("TRN2", target_bir_lowering=False, debug=False,
                   num_devices=N_CORES)

    hencT = nc.dram_tensor("hencT", [BL, ND, 128, S], F32R, kind="ExternalInput").ap()
    hencN = nc.dram_tensor("hencN", [BL, NS, 128, D], F32R, kind="ExternalInput").ap()
    wenc = nc.dram_tensor("wenc", [ND, 128, U], F32R, kind="ExternalInput").ap()
    wdec = nc.dram_tensor("wdec", [ND, 128, U], F32, kind="ExternalInput").ap()
    hdT = nc.dram_tensor("hdT", [128, ND, BL], F32, kind="ExternalInput").ap()
    bencT = nc.dram_tensor("bencT", [128, NU], F32, kind="ExternalInput").ap()
    bdecT = nc.dram_tensor("bdecT", [128, NU], F32, kind="ExternalInput").ap()
    bencN = nc.dram_tensor("bencN", [1, U], F32, kind="ExternalInput").ap()
    wcomT = nc.dram_tensor("wcomT", [128, NU], F32R, kind="ExternalInput").ap()

    ctx_out = nc.dram_tensor("ctx_out", [BL, U], F32, kind="ExternalOutput").ap()
    attn_out = nc.dram_tensor("attn_out", [BL, S], F32, kind="ExternalOutput").ap()

    with tile.TileContext(nc) as tc:
        with (
            tc.tile_pool(name="const", bufs=1) as constp,
            tc.tile_pool(name="hT", bufs=3) as hTp,
            tc.tile_pool(name="tT", bufs=4) as tTp,
            tc.tile_pool(name="hn", bufs=8) as hnp,
            tc.tile_pool(name="row", bufs=2) as rowp,
            tc.tile_pool(name="small", bufs=4) as smallp,
            tc.tile_pool(name="mm", bufs=2, space="PSUM") as mmp,
            tc.tile_pool(name="vec", bufs=5, space="PSUM") as vecp,
            tc.tile_pool(name="tp", bufs=1, space="PSUM") as tpp,
        ):
            # ---------- constants ----------
            wenc_sb = constp.tile([128, ND, U], F32R, tag="wenc")
            for k in range(ND):
                nc.sync.dma_start(out=wenc_sb[:, k, :], in_=wenc[k])
            wdec_sb = constp.tile([128, ND, U], F32, tag="wdec")
            for k in range(ND):
                nc.sync.dma_start(out=wdec_sb[:, k, :], in_=wdec[k])
            hdT_sb = constp.tile([128, ND, BL], F32, tag="hdT")
            nc.sync.dma_start(out=hdT_sb[:], in_=hdT[:])
            bencT_sb = constp.tile([128, NU], F32, tag="bencT")
            nc.sync.dma_start(out=bencT_sb[:], in_=bencT[:])
            bdecT_sb = constp.tile([128, NU], F32, tag="bdecT")
            nc.sync.dma_start(out=bdecT_sb[:], in_=bdecT[:])
            bencN_sb = constp.tile([1, U], F32, tag="bencN")
            nc.sync.dma_start(out=bencN_sb[:], in_=bencN[:])
            wcomT_sb = constp.tile([128, NU], F32R, tag="wcomT")
            nc.sync.dma_start(out=wcomT_sb[:], in_=wcomT[:])
            ones_sb = constp.tile([1, 1], F32, tag="ones")
            nc.vector.memset(ones_sb[:], 1.0)

            # ---------- dec projection: zT[u, j, b] = dec_p^T + b_dec + b_enc ----------
            zT_sb = constp.tile([128, NU, BL], F32, tag="zT")
            for j in range(NU):
                dp_ps = tpp.tile([128, BL], F32, tag="tp")
                for k in range(ND):
                    nc.tensor.matmul(
                        dp_ps[:],
                        wdec_sb[:, k, j * 128:(j + 1) * 128],
                        hdT_sb[:, k, :],
                        start=(k == 0), stop=(k == ND - 1),
                    )
                nc.vector.tensor_scalar(
                    zT_sb[:, j, :], dp_ps[:],
                    bdecT_sb[:, j:j + 1], bencT_sb[:, j:j + 1],
                    ALU.add, ALU.add,
                )

            for b in range(BL):
                # ---------- phase A: enc proj + tanh + score ----------
                score_sb = rowp.tile([1, S], F32, tag="row")
                for c in range(NSC):
                    hT_c = hTp.tile([128, ND, SC], F32R, tag="hT")
                    for k in range(ND):
                        nc.sync.dma_start(
                            out=hT_c[:, k, :],
                            in_=hencT[b, k, :, c * SC:(c + 1) * SC],
                        )
                    score_ps = vecp.tile([1, SC], F32, tag="vec")
                    for j in range(NU):
                        mm_ps = mmp.tile([128, SC], F32, tag="mm")
                        for k in range(ND):
                            nc.tensor.matmul(
                                mm_ps[:],
                                wenc_sb[:, k, j * 128:(j + 1) * 128],
                                hT_c[:, k, :],
                                start=(k == 0), stop=(k == ND - 1),
                            )
                        tT = tTp.tile([128, SC], F32R, tag="tT")
                        nc.scalar.activation(tT[:], mm_ps[:], AF.Tanh,
                                             bias=zT_sb[:, j, b:b + 1])
                        nc.tensor.matmul(
                            score_ps[:],
                            wcomT_sb[:, j:j + 1],
                            tT[:],
                            start=(j == 0), stop=(j == NU - 1),
                        )
                    nc.vector.tensor_copy(score_sb[:, c * SC:(c + 1) * SC], score_ps[:])

                # ---------- phase B: softmax over S (partition 0) ----------
                negmax = smallp.tile([1, 1], F32, tag="scalars")
                nc.vector.tensor_reduce(negmax[:], score_sb[:], AX.X, ALU.max,
                                        negate=True)
                exp_sb = rowp.tile([1, S], F32, tag="row")
                sumexp = smallp.tile([1, 1], F32, tag="scalars")
                nc.scalar.activation(exp_sb[:], score_sb[:], AF.Exp,
                                     bias=negmax[:], accum_out=sumexp[:])
                inv = smallp.tile([1, 1], F32, tag="scalars")
                nc.vector.reciprocal(inv[:], sumexp[:])
                attn_sb = rowp.tile([1, S], F32, tag="row")
                nc.vector.tensor_scalar(attn_sb[:], exp_sb[:], inv[:], None, ALU.mult)
                nc.sync.dma_start(out=attn_out[b:b + 1, :], in_=attn_sb[:])

                # ---------- phase C: exp row -> columns via K=1 matmul ----------
                acol_ps = tpp.tile([128, NS], F32, tag="tp")
                for i in range(NS):
                    nc.tensor.matmul(
                        acol_ps[:, i:i + 1],
                        exp_sb[:, i * 128:(i + 1) * 128],
                        ones_sb[:],
                        start=True, stop=True,
                    )
                acol_sb = smallp.tile([128, NS], F32R, tag="acol")
                nc.vector.tensor_copy(acol_sb[:], acol_ps[:])

                # ---------- phase D: ctx_pre = exp^T @ h ----------
                ctx_ps = [vecp.tile([1, 512], F32, tag="vec", name=f"ctx_ps{b}_{c2}")
                          for c2 in range(2)]
                for i in range(NS):
                    hn = hnp.tile([128, D], F32R, tag="hn")
                    nc.sync.dma_start(out=hn[:], in_=hencN[b, i])
                    for c2 in range(2):
                        nc.tensor.matmul(
                            ctx_ps[c2][:],
                            acol_sb[:, i:i + 1],
                            hn[:, c2 * 512:(c2 + 1) * 512],
                            start=(i == 0), stop=(i == NS - 1),
                        )

                # ---------- phase E: scale, transpose, final projection ----------
                ctxs_sb = rowp.tile([1, D], F32, tag="rowD")
                for c2 in range(2):
                    nc.vector.tensor_scalar(
                        ctxs_sb[:, c2 * 512:(c2 + 1) * 512], ctx_ps[c2][:],
                        inv[:], None, ALU.mult,
                    )
                ctxT_ps = tpp.tile([128, ND], F32, tag="tp")
                for j in range(ND):
                    nc.tensor.matmul(
                        ctxT_ps[:, j:j + 1],
                        ctxs_sb[:, j * 128:(j + 1) * 128],
                        ones_sb[:],
                        start=True, stop=True,
                    )
                ctxT_sb = smallp.tile([128, ND], F32R, tag="ctxT")
                nc.vector.tensor_copy(ctxT_sb[:], ctxT_ps[:])
                for c2 in range(2):
                    fin_ps = vecp.tile([1, 512], F32, tag="vec")
                    for j in range(ND):
                        nc.tensor.matmul(
                            fin_ps[:],
                            ctxT_sb[:, j:j + 1],
                            wenc_sb[:, j, c2 * 512:(c2 + 1) * 512],
                            start=(j == 0), stop=(j == ND - 1),
                        )
                    out_sb = smallp.tile([1, 512], F32, tag="outrow")
                    nc.vector.tensor_tensor(out_sb[:], fin_ps[:],
                                            bencN_sb[:, c2 * 512:(c2 + 1) * 512],
                                            ALU.add)
                    nc.sync.dma_start(out=ctx_out[b:b + 1, c2 * 512:(c2 + 1) * 512],
                                      in_=out_sb[:])

    nc.compile()
    return nc


_program = None


def _get_program():
    global _program
    if _program is None:
        _program = build_program()
    return _program


def make_in_maps(h_enc, h_dec, W_enc, b_enc, W_dec, b_dec, W_com, b_com):
    """Shard + layout-rearrange the full inputs into per-core input maps."""
    h_enc = np.asarray(h_enc, dtype=np.float32)
    h_dec = np.asarray(h_dec, dtype=np.float32)
    W_enc = np.asarray(W_enc, dtype=np.float32)
    b_enc = np.asarray(b_enc, dtype=np.float32)
    W_dec = np.asarray(W_dec, dtype=np.float32)
    b_dec = np.asarray(b_dec, dtype=np.float32)
    W_com = np.asarray(W_com, dtype=np.float32)

    wenc_a = np.ascontiguousarray(W_enc.reshape(ND, 128, U))
    wdec_a = np.ascontiguousarray(W_dec.reshape(ND, 128, U))
    bencT_a = np.ascontiguousarray(b_enc.reshape(NU, 128).T)
    bdecT_a = np.ascontiguousarray(b_dec.reshape(NU, 128).T)
    bencN_a = np.ascontiguousarray(b_enc.reshape(1, U))
    wcomT_a = np.ascontiguousarray(W_com[:, 0].reshape(NU, 128).T)

    in_maps = []
    for core in range(N_CORES):
        hb = h_enc[core * BL:(core + 1) * BL]            # [BL, S, D]
        hdb = h_dec[core * BL:(core + 1) * BL]           # [BL, D]
        hencT_a = np.ascontiguousarray(
            hb.transpose(0, 2, 1).reshape(BL, ND, 128, S))
        hencN_a = np.ascontiguousarray(hb.reshape(BL, NS, 128, D))
        hdT_a = np.ascontiguousarray(
            hdb.reshape(BL, ND, 128).transpose(2, 1, 0))  # [128, ND, BL]
        in_maps.append({
            "hencT": hencT_a,
            "hencN": hencN_a,
            "wenc": wenc_a,
            "wdec": wdec_a,
            "hdT": hdT_a,
            "bencT": bencT_a,
            "bdecT": bdecT_a,
            "bencN": bencN_a,
            "wcomT": wcomT_a,
        })
    return in_maps


def kernel(h_enc, h_dec, W_enc, b_enc, W_dec, b_dec, W_com, b_com,
           _trace=False, _tmpdir=None):
    nc = _get_program()
    in_maps = make_in_maps(h_enc, h_dec, W_enc, b_enc, W_dec, b_dec,
                           W_com, b_com)
    res = run_bass_kernel_spmd(nc, in_maps, list(range(N_CORES)),
                               trace=_trace, tmpdir=_tmpdir)
    ctx = np.concatenate([res.results[i]["ctx_out"] for i in range(N_CORES)],
                         axis=0)
    attn = np.concatenate([res.results[i]["attn_out"] for i in range(N_CORES)],
                          axis=0).reshape(B, S, 1)
    if _trace:
        kernel.last_results = res
    return ctx.astype(np.float32), attn.astype(np.float32)
